# revision 32
# baseline (speedup 1.0000x reference)
"""Trainium2 Bass kernel for CodecAttention (GQA + full-width RMSNorm + ALiBi
+ 512 sliding causal window), SPMD over 8 NeuronCores.

Sharding: 2 batches x 4 sequence chunks of 512 queries per core. Each core
sees a [q0-512, q0+512) feature slice (zero-padded left halo for chunk 0),
computes its own QKV projections + norms + banded windowed attention + output
projection. Host only slices/transposes/casts inputs and concatenates outputs.

v3 structure:
- bf16 operands for projections / PV / out-proj; f32r for attention scores
  (the in-matmul bias rows need >8 mantissa bits for exact key positions).
- ALiBi bias slope*(pos-512), the per-query exponent correction -slope*q and
  the chunk-0 halo kill are all folded into the score matmul via two spare
  contraction rows (K-side: pos-512 / ones; Q-side: 8*slope / -8*slope*q).
  The exp activation is then bias-free, so score tiles for one head are
  packed into double-buffered PSUM chunks (2+2+1 banks, zero garbage) and
  exponentiated with three large ACT calls per head.
- rmsnorm weights are folded into wq/wk host-side; all row broadcasts
  (rms reciprocals, softmax denominators) go through gpsimd
  partition_broadcast instead of PE matmuls.
- Projection staging copies are split between the scalar and vector engines;
  triangle masks alternate between gpsimd affine_select and DVE mask-muls
  so no single engine serializes the attention inner loop.
"""

import math

import numpy as np
import ml_dtypes

import concourse.bass as bass
import concourse.tile as tile
from concourse import bacc, mybir

F32 = mybir.dt.float32
F32R = mybir.dt.float32r
BF16 = mybir.dt.bfloat16
AF = mybir.ActivationFunctionType
NPBF16 = ml_dtypes.bfloat16

# Problem constants (hardcoded per spec nn_CodecAttention_34308198761010)
B, S, M = 2, 2048, 1024
H, KV, D = 16, 4, 64
WIN = 512
SQ = 512          # queries per core
SK = 1024         # k-range per core (halo + chunk)
NCHUNK = S // SQ  # 4
N_CORES = 8
EPS = 1e-6
# Head order permutation (host-side, applied to wq/q_norm_w/wo): Q-head slot
# 2j+sub sits at partition offset 64*sub, which must equal its KV head's
# partition offset 64*((h//4)%2) (matmul requires equal base partitions).
PERM = [0, 4, 1, 5, 2, 6, 3, 7, 8, 12, 9, 13, 10, 14, 11, 15]

# banded spans: for k-tile kt (128 rows of kl), valid q columns [QL[kt], QR[kt])
QL = [0, 0, 0, 0, 0, 128, 256, 384]
QR = [128, 256, 384, 512, 512, 512, 512, 512]

# Score-band PSUM chunks (each (kt, col offset); every matmul output inside
# one 512-float bank, zero garbage columns):
#   C2A = 2 banks {kt3, kt4}, C2B = 2 banks {kt2, kt0, kt5, kt7},
#   C1 = 1 bank {kt1, kt6}
CHUNK_C2A = [(3, 0), (4, 512)]
CHUNK_C2B = [(2, 0), (0, 384), (5, 512), (7, 896)]
CHUNK_C1 = [(1, 0), (6, 256)]
# diagonal-block masks: (col offset, type, engine) per chunk.
# type 1: keep col' <= p (window bound, kt<=3); type 2: keep col' >= p
# (causal bound, kt>=4). engine 0 = gpsimd affine_select, 1 = DVE mask-mul.
MASKS_C2A = [(384, 1, 0), (512, 2, 1)]
MASKS_C2B = [(256, 1, 0), (384, 1, 0), (512, 2, 0), (896, 2, 0)]
MASKS_C1 = [(128, 1, 0), (256, 2, 1)]


def _alibi_slopes(n):
    ratio = 2.0 ** (-8.0 / n)
    return np.array([ratio**i for i in range(n)], dtype=np.float64)


def build_nc(for_sim=False):
    nc = bacc.Bacc(None, target_bir_lowering=False, debug=for_sim)

    feat_d = nc.dram_tensor("feat", [128, 8, SK], BF16, kind="ExternalInput")
    wq_d = nc.dram_tensor("wq", [8, 128, 8, 128], BF16, kind="ExternalInput")
    wk_d = nc.dram_tensor("wk", [128, 8, 256], BF16, kind="ExternalInput")
    wv_d = nc.dram_tensor("wv", [128, 8, 256], BF16, kind="ExternalInput")
    wo_d = nc.dram_tensor("wo", [128, 8, M], BF16, kind="ExternalInput")
    ones_d = nc.dram_tensor("onesin", [128, 128], F32R, kind="ExternalInput")
    sel_d = nc.dram_tensor("sel", [128, 2, 128], F32R, kind="ExternalInput")
    tri_d = nc.dram_tensor("tri", [128, 2, 128], BF16, kind="ExternalInput")
    vone_d = nc.dram_tensor("vone", [128, 8, KV, 1], BF16,
                            kind="ExternalInput")
    kbias_d = nc.dram_tensor("kbias", [3, 2, SK], BF16, kind="ExternalInput")
    qbias_d = nc.dram_tensor("qbias", [2, 3, 8, SQ], BF16,
                             kind="ExternalInput")
    out_d = nc.dram_tensor("out", [SQ, M], F32, kind="ExternalOutput")

    with tile.TileContext(nc) as tc:
        with (
            nc.allow_low_precision("low precision matmul operands are "
                                   "intentional"),
            tc.tile_pool(name="const", bufs=1) as constp,
            tc.tile_pool(name="acts", bufs=1) as actsp,
            tc.tile_pool(name="se2a", bufs=2) as se2ap,
            tc.tile_pool(name="se2b", bufs=2) as se2bp,
            tc.tile_pool(name="se1", bufs=2) as se1p,
            tc.tile_pool(name="sqpool", bufs=2) as sqp,
            tc.tile_pool(name="small", bufs=1) as smallp,
            tc.tile_pool(name="outsb", bufs=2) as outp,
        ):
            # ---- shared constant tiles (DMAs issued after feat below) ----
            ones_sb = constp.tile([128, 128], F32R)
            sel_sb = constp.tile([128, 2, 128], F32R)
            tri_sb = constp.tile([128, 2, 128], BF16)
            eps_sb = constp.tile([128, 1], F32)
            nc.vector.memset(eps_sb[:], EPS)

            # long-lived activations.
            # kt_sb[:, par, gi, :]: kv head g=2*gi+par; data rows 64*par..+64,
            # bias rows (pos-512, ones) in the opposite half, rest zero.
            # qt_sb[:, par, i, :]: head PERM[2*i+par]; same row scheme with
            # Q-side bias rows (8*slope, -8*slope*q).
            kt_sb = actsp.tile([128, 2, 2, SK], BF16)
            qt_sb = actsp.tile([128, 2, 8, SQ], BF16)
            # v_sb cols: [0:64 V][64 ones]: PV rows 0:64, denom row 64
            v_sb = actsp.tile([128, 8, KV, 65], BF16)
            attn_sb = actsp.tile([128, 8, SQ], BF16)

            # ================= projection phase (scoped SBUF) ==========
            with (
                tc.tile_pool(name="wkv", bufs=1) as wkvp,
                tc.tile_pool(name="feat", bufs=1) as featp,
                tc.tile_pool(name="wstream", bufs=2) as wsp,
            ):
                wk_sb = wkvp.tile([128, 8, 256], BF16)
                wv_sb = wkvp.tile([128, 8, 256], BF16)
                feat_sb = featp.tile([128, 8, SK], BF16)
                nc.sync.dma_start(wk_sb[:], wk_d[:])
                for mt in range(8):
                    nc.sync.dma_start(feat_sb[:, mt, :], feat_d[:, mt, :])
                nc.sync.dma_start(wv_sb[:], wv_d[:])
                nc.sync.dma_start(ones_sb[:], ones_d[:])
                nc.sync.dma_start(sel_sb[:], sel_d[:])
                nc.sync.dma_start(tri_sb[:], tri_d[:])
                nc.sync.dma_start(v_sb[:, :, :, 64:65], vone_d[:])
                # zero-init tiles whose unwritten rows feed sel-broadcast
                # matmuls (only one row carries data; the sel matrix zeroes
                # the rest, but they must hold finite values)
                srtk = smallp.tile([128, SK], F32R, tag="srtk", name="srtk")
                srtq = smallp.tile([128, SQ], F32R, tag="srtq", name="srtq")
                den0 = smallp.tile([128, 2, SQ], F32R, tag="den", name="den")
                nc.vector.tensor_scalar_mul(srtk[:], feat_sb[:, 0, :], 0.0)
                nc.vector.tensor_scalar_mul(srtq[:], feat_sb[:, 0, 0:512],
                                            0.0)
                nc.vector.tensor_scalar_mul(
                    den0[:].rearrange("p a b -> p (a b)"),
                    feat_sb[:, 1, :], 0.0)
                # zero the spare halves of kt/qt once while DMAs stream in
                # (src operand is only for shape; value is ignored for *0.0)
                nc.vector.tensor_scalar_mul(
                    kt_sb[64:128, 0, :, :].rearrange("p a b -> p (a b)"),
                    feat_sb[64:128, 0:2, :].rearrange("p a b -> p (a b)"),
                    0.0)
                nc.vector.tensor_scalar_mul(
                    kt_sb[0:64, 1, :, :].rearrange("p a b -> p (a b)"),
                    feat_sb[0:64, 0:2, :].rearrange("p a b -> p (a b)"),
                    0.0)
                nc.vector.tensor_scalar_mul(
                    qt_sb[64:128, 0, :, :].rearrange("p a b -> p (a b)"),
                    feat_sb[64:128, 0:4, :].rearrange("p a b -> p (a b)"),
                    0.0)
                nc.vector.tensor_scalar_mul(
                    qt_sb[0:64, 1, :, :].rearrange("p a b -> p (a b)"),
                    feat_sb[0:64, 0:4, :].rearrange("p a b -> p (a b)"),
                    0.0)

                # ---- K/V/Q projections: V-pool outer so V matmuls fill
                # the gaps in K's rmsnorm tail; ssq/broadcast tiles rotate
                # through one small pool to stay inside 8 PSUM banks ----
                with tc.tile_pool(name="psV", bufs=2,
                                  space=bass.MemorySpace.PSUM) as psV:
                    with (
                        tc.tile_pool(name="psK", bufs=1,
                                     space=bass.MemorySpace.PSUM) as psK,
                        tc.tile_pool(name="psKN", bufs=1,
                                     space=bass.MemorySpace.PSUM) as psKN,
                    ):
                        kp = [psK.tile([128, SK], F32, tag=f"kp{t}",
                                       name=f"kp{t}")
                              for t in range(2)]
                        # mt-major so matmuls start as soon as each feature
                        # slice lands
                        for mt in range(8):
                            for t in range(2):
                                for half in range(2):
                                    cs = slice(512 * half, 512 * half + 512)
                                    nc.tensor.matmul(
                                        kp[t][:, cs],
                                        wk_sb[:, mt, 128 * t:128 * t + 128],
                                        feat_sb[:, mt, cs],
                                        start=(mt == 0), stop=(mt == 7))
                        ssqk = psKN.tile([128, SK], F32, tag="kn",
                                         name="ssqk")
                        for t in range(2):
                            sqk = sqp.tile([128, SK], F32R, tag="sqk")
                            nc.scalar.activation(sqk[:], kp[t][:], AF.Square)
                            for half in range(2):
                                cs = slice(512 * half, 512 * half + 512)
                                nc.tensor.matmul(ssqk[0:1, cs],
                                                 ones_sb[:, 0:1],
                                                 sqk[:, cs],
                                                 start=(t == 0),
                                                 stop=(t == 1))
                        nc.scalar.activation(srtk[0:1, :], ssqk[0:1, :],
                                             AF.Sqrt, scale=1.0 / 256.0,
                                             bias=eps_sb[0:1, :])
                        bck = psKN.tile([128, SK], F32, tag="kn",
                                        name="bck")
                        for half in range(2):
                            cs = slice(512 * half, 512 * half + 512)
                            nc.tensor.matmul(bck[:, cs], sel_sb[:, 0, :],
                                             srtk[:, cs], start=True,
                                             stop=True)
                        bcki = smallp.tile([128, SK], F32, tag="bcki")
                        nc.vector.reciprocal_approx_fast(bcki[:], bck[:])
                        # fused stage+norm: kt = kp * (1/rms) from PSUM
                        for t in range(2):
                            nc.vector.tensor_mul(kt_sb[0:64, 0, t, :],
                                                 kp[t][0:64, :],
                                                 bcki[0:64, :])
                            nc.vector.tensor_mul(kt_sb[64:128, 1, t, :],
                                                 kp[t][64:128, :],
                                                 bcki[64:128, :])

                        # ---- V projection (fills K norm-chain gaps) ----
                        for st in range(8):
                            vp = psV.tile([128, 256], F32, tag="vp",
                                          name="vp")
                            for mt in range(8):
                                nc.tensor.matmul(
                                    vp[:],
                                    feat_sb[:, mt, 128 * st:128 * st + 128],
                                    wv_sb[:, mt, :],
                                    start=(mt == 0), stop=(mt == 7))
                            nc.vector.tensor_copy(
                                v_sb[:, st, :, 0:64],
                                vp[:].rearrange("p (g d) -> p g d", g=KV))

                    # ---- Q projection + rmsnorm ----
                    with (
                        tc.tile_pool(name="psQ", bufs=2,
                                     space=bass.MemorySpace.PSUM) as psQ,
                        tc.tile_pool(name="psQN", bufs=1,
                                     space=bass.MemorySpace.PSUM) as psQN,
                    ):
                        ssqq = psQN.tile([128, SQ], F32, tag="qn",
                                         name="ssqq")
                        for t in range(8):
                            wqt = wsp.tile([128, 8, 128], BF16, tag="wqt")
                            nc.sync.dma_start(wqt[:], wq_d[t])
                            qp = psQ.tile([128, SQ], F32)
                            for mt in range(8):
                                nc.tensor.matmul(qp[:], wqt[:, mt, :],
                                                 feat_sb[:, mt, 512:1024],
                                                 start=(mt == 0),
                                                 stop=(mt == 7))
                            sqq = sqp.tile([128, SQ], F32R, tag="sqq")
                            nc.scalar.activation(sqq[:], qp[:], AF.Square)
                            nc.tensor.matmul(ssqq[0:1, :], ones_sb[:, 0:1],
                                             sqq[:],
                                             start=(t == 0), stop=(t == 7))
                            # staging copies split across scalar/vector
                            nc.scalar.copy(qt_sb[0:64, 0, t, :], qp[0:64, :])
                            nc.vector.tensor_copy(qt_sb[64:128, 1, t, :],
                                                  qp[64:128, :])
                        nc.scalar.activation(srtq[0:1, :], ssqq[0:1, :],
                                             AF.Sqrt, scale=1.0 / 1024.0,
                                             bias=eps_sb[0:1, :])
                        bcq = psQN.tile([128, SQ], F32, tag="qn",
                                        name="bcq")
                        nc.tensor.matmul(bcq[:], sel_sb[:, 0, :],
                                         srtq[:, :], start=True, stop=True)
                        bcqi = smallp.tile([128, SQ], F32, tag="bcqi")
                        nc.vector.reciprocal_approx_fast(bcqi[:], bcq[:])
                        for t in range(8):
                            nc.vector.tensor_mul(qt_sb[0:64, 0, t, :],
                                                 qt_sb[0:64, 0, t, :],
                                                 bcqi[0:64, :])
                            nc.vector.tensor_mul(qt_sb[64:128, 1, t, :],
                                                 qt_sb[64:128, 1, t, :],
                                                 bcqi[64:128, :])

            # ================= attention phase ========================
            # bias rows overwrite two rows of the zeroed spare halves
            nc.sync.dma_start(kt_sb[64:67, 0, :, :], kbias_d[:])
            nc.sync.dma_start(kt_sb[0:3, 1, :, :], kbias_d[:])
            nc.sync.dma_start(qt_sb[64:67, 0, :, :], qbias_d[0])
            nc.sync.dma_start(qt_sb[0:3, 1, :, :], qbias_d[1])
            with (
                tc.tile_pool(name="wbig", bufs=1) as wbigp,
            ):
                wo_sb = wbigp.tile([128, 8, M], BF16)
                for c in range(2):
                    nc.sync.dma_start(wo_sb[:, 4 * c:4 * c + 4, :],
                                      wo_d[:, 4 * c:4 * c + 4, :])

                with (
                    tc.tile_pool(name="psC2", bufs=2,
                                 space=bass.MemorySpace.PSUM) as psC2,
                    tc.tile_pool(name="psC1", bufs=1,
                                 space=bass.MemorySpace.PSUM) as psC1,
                    tc.tile_pool(name="psPV", bufs=1,
                                 space=bass.MemorySpace.PSUM) as psPV,
                    tc.tile_pool(name="psBC", bufs=1,
                                 space=bass.MemorySpace.PSUM) as psBC,
                ):
                    pv_pair = [None, None]
                    for s in range(16):
                        par, idx = s % 2, s // 2
                        h = PERM[s]
                        g = h // 4
                        assert g % 2 == par
                        gi = g // 2
                        pv = psPV.tile([128, SQ], F32, tag=f"pv{s % 2}",
                                       name=f"pv{s % 2}")
                        pv_pair[par] = pv

                        ps2a = psC2.tile([128, 1024], F32, tag="c2",
                                         name="ps2a")
                        ps2b = psC2.tile([128, 1024], F32, tag="c2",
                                         name="ps2b")
                        ps1 = psC1.tile([128, 512], F32, tag="c1",
                                        name="ps1")
                        chunks = ((CHUNK_C2A, ps2a), (CHUNK_C2B, ps2b),
                                  (CHUNK_C1, ps1))
                        for chunk, ps in chunks:
                            for kt, off in chunk:
                                span = QR[kt] - QL[kt]
                                nc.tensor.matmul(
                                    ps[:, off:off + span],
                                    kt_sb[:, par, gi,
                                          128 * kt:128 * kt + 128],
                                    qt_sb[:, par, idx, QL[kt]:QR[kt]],
                                    start=True, stop=True)
                        se2a = se2ap.tile([128, 1024], BF16, tag="se2a")
                        se2b = se2bp.tile([128, 1024], BF16, tag="se2b")
                        se1 = se1p.tile([128, 512], BF16, tag="se1")
                        semap = ((se2a, ps2a, MASKS_C2A),
                                 (se2b, ps2b, MASKS_C2B),
                                 (se1, ps1, MASKS_C1))
                        for se, ps, masks in semap:
                            nc.scalar.activation(se[:], ps[:], AF.Exp,
                                                 scale=0.125)
                            for off, mtype, eng in masks:
                                blk = se[:, off:off + 128]
                                if eng == 0:
                                    if mtype == 1:  # keep col' <= p
                                        nc.gpsimd.affine_select(
                                            blk, blk, pattern=[[-1, 128]],
                                            compare_op=mybir.AluOpType.is_ge,
                                            fill=0.0, base=0,
                                            channel_multiplier=1)
                                    else:  # keep col' >= p
                                        nc.gpsimd.affine_select(
                                            blk, blk, pattern=[[1, 128]],
                                            compare_op=mybir.AluOpType.is_ge,
                                            fill=0.0, base=0,
                                            channel_multiplier=-1)
                                else:
                                    nc.vector.tensor_mul(
                                        blk, blk,
                                        tri_sb[:, mtype - 1, :])
                        nmm = 0
                        sechunks = ((CHUNK_C2A, se2a), (CHUNK_C2B, se2b),
                                    (CHUNK_C1, se1))
                        for chunk, se in sechunks:
                            for kt, off in chunk:
                                span = QR[kt] - QL[kt]
                                nc.tensor.matmul(
                                    pv[0:65, QL[kt]:QR[kt]],
                                    v_sb[:, kt, g, 0:65],
                                    se[:, off:off + span],
                                    start=(nmm == 0), stop=(nmm == 7))
                                nmm += 1

                        if par == 1:
                            # finalize pair: attn = pv / denom per head
                            # (reciprocal runs after the broadcast on a
                            # [64, SQ] tile -- single-partition custom-DVE
                            # reciprocals misbehave on hardware)
                            p_i = idx
                            den = den0
                            nc.vector.tensor_copy(den[64:65, 0, :],
                                                  pv_pair[0][64:65, :])
                            nc.vector.tensor_copy(den[64:65, 1, :],
                                                  pv_pair[1][64:65, :])
                            bc0 = psBC.tile([64, SQ], F32, tag="bc",
                                            name="bc0")
                            nc.tensor.matmul(bc0[:], sel_sb[:, 1, 0:64],
                                             den[:, 0, :],
                                             start=True, stop=True)
                            bci0 = smallp.tile([64, SQ], F32, tag="bci0")
                            nc.vector.reciprocal_approx_fast(bci0[:],
                                                             bc0[:])
                            nc.vector.tensor_mul(attn_sb[0:64, p_i, :],
                                                 pv_pair[0][0:64, :],
                                                 bci0[:])
                            bc1 = psBC.tile([64, SQ], F32, tag="bc",
                                            name="bc1")
                            nc.tensor.matmul(bc1[:], sel_sb[:, 1, 0:64],
                                             den[:, 1, :],
                                             start=True, stop=True)
                            bci1 = smallp.tile([64, SQ], F32, tag="bci1")
                            nc.vector.reciprocal_approx_fast(bci1[:],
                                                             bc1[:])
                            atmp = smallp.tile([64, SQ], BF16, tag="atmp")
                            nc.vector.tensor_mul(atmp[:],
                                                 pv_pair[1][0:64, :],
                                                 bci1[:])
                            nc.sync.dma_start(attn_sb[64:128, p_i, :],
                                              atmp[:])

                # ---- output projection (wo still resident) ----
                outv = out_d.rearrange("(st p) m -> st p m", p=128)
                with tc.tile_pool(name="psO", bufs=4,
                                  space=bass.MemorySpace.PSUM) as psO:
                    for st in range(4):
                        osb = outp.tile([128, M], F32, tag="osb")
                        for mh in range(2):
                            op = psO.tile([128, 512], F32)
                            for ht in range(8):
                                nc.tensor.matmul(
                                    op[:],
                                    attn_sb[:, ht, 128 * st:128 * st + 128],
                                    wo_sb[:, ht, 512 * mh:512 * mh + 512],
                                    start=(ht == 0), stop=(ht == 7))
                            if mh == 0:
                                nc.scalar.copy(
                                    osb[:, 512 * mh:512 * mh + 512], op[:])
                            else:
                                nc.vector.tensor_copy(
                                    osb[:, 512 * mh:512 * mh + 512], op[:])
                        nc.sync.dma_start(outv[st], osb[:])

    if for_sim:
        nc.compile()
    else:
        nc.finalize()
    return nc


def make_in_maps(features, wq, wk, wv, wo, q_norm_w, k_norm_w):
    features = np.asarray(features, np.float32)
    wq = np.asarray(wq, np.float32)
    wk = np.asarray(wk, np.float32)
    wv = np.asarray(wv, np.float32)
    wo = np.asarray(wo, np.float32)
    q_norm_w = np.asarray(q_norm_w, np.float32)
    k_norm_w = np.asarray(k_norm_w, np.float32)

    # permute Q-head order (see PERM) in wq rows, q_norm_w, wo columns;
    # fold the rmsnorm weights into the projection rows (commutes with the
    # per-position rms scaling)
    wq_p = wq.reshape(H, D, M)[PERM].reshape(H * D, M)
    qnw_p = q_norm_w.reshape(H, D)[PERM].reshape(H * D)
    wq_p = wq_p * qnw_p[:, None]
    wk_f = wk * k_norm_w[:, None]
    wo_tp = wo.T.reshape(H, D, M)[PERM].reshape(H * D, M)  # wo.T rows = hd

    wq_pre = np.ascontiguousarray(
        wq_p.T.reshape(8, 128, 8, 128).transpose(2, 1, 0, 3)).astype(NPBF16)
    wk_pre = np.ascontiguousarray(
        wk_f.T.reshape(8, 128, 256).transpose(1, 0, 2)).astype(NPBF16)
    wv_pre = np.ascontiguousarray(
        wv.T.reshape(8, 128, 256).transpose(1, 0, 2)).astype(NPBF16)
    wo_pre = np.ascontiguousarray(
        wo_tp.reshape(8, 128, M).transpose(1, 0, 2)).astype(NPBF16)

    slopes = _alibi_slopes(H)

    vone = np.ones((128, 8, KV, 1), NPBF16)
    # row selectors: sel[:,0,:] picks partition 0, sel[:,1,:] picks 64
    sel = np.zeros((128, 2, 128), np.float32)
    sel[0, 0, :] = 1.0
    sel[64, 1, :] = 1.0
    # triangle mask tiles for the DVE mask-muls: tri[:,0]=keep col<=p,
    # tri[:,1]=keep col>=p
    p = np.arange(128)
    tri = np.zeros((128, 2, 128), NPBF16)
    tri[:, 0, :] = (p[None, :] <= p[:, None])
    tri[:, 1, :] = (p[None, :] >= p[:, None])

    # Q-side bias rows: rows 0/1 = 8*slope_h (pair with K-side pos_hi and
    # pos_lo; the position is split so both parts are bf16-exact),
    # row 2 = -8*slope_h*q - 320 (pairs with K-side ones; cancels in
    # softmax, keeps exp args <= ~-28 valid / < ~65 in masked triangles so
    # nothing overflows to inf before the mask-muls)
    qi = np.arange(SQ, dtype=np.float64)
    qbias = np.zeros((2, 3, 8, SQ), np.float32)
    for s in range(16):
        par, idx = s % 2, s // 2
        sl = slopes[PERM[s]]
        qbias[par, 0, idx, :] = 8.0 * sl
        qbias[par, 1, idx, :] = 8.0 * sl
        qbias[par, 2, idx, :] = -8.0 * sl * qi - 320.0
    qbias = qbias.astype(NPBF16)

    in_maps = []
    for b in range(B):
        for c in range(NCHUNK):
            q0 = c * SQ
            lo, hi = q0 - WIN, q0 + SQ
            fs = np.zeros((SK, M), np.float32)
            src_lo = max(lo, 0)
            fs[src_lo - lo:, :] = features[b, src_lo:hi, :]
            feat_pre = np.ascontiguousarray(
                fs.T.reshape(8, 128, SK).transpose(1, 0, 2)).astype(NPBF16)
            # K-side bias rows: rows 0/1 = pos_hi/pos_lo with
            # pos_hi + pos_lo = pos-512, both bf16-exact (halo positions
            # get a huge negative value so exp underflows to 0); row 2 = 1
            kbias = np.zeros((3, 2, SK), np.float32)
            pos = np.arange(SK, dtype=np.float64) - 512.0
            pos_hi = 4.0 * np.floor(pos / 4.0)
            pos_lo = pos - pos_hi
            if c == 0:
                pos_hi[:512] = -1e30
                pos_lo[:512] = 0.0
            kbias[0, :, :] = pos_hi[None, :]
            kbias[1, :, :] = pos_lo[None, :]
            kbias[2, :, :] = 1.0
            kbias = kbias.astype(NPBF16)
            in_maps.append({
                "feat": feat_pre, "wq": wq_pre, "wk": wk_pre, "wv": wv_pre,
                "wo": wo_pre,
                "onesin": np.ones((128, 128), np.float32),
                "sel": sel, "tri": tri, "vone": vone,
                "kbias": kbias, "qbias": qbias,
            })
    return in_maps


_NC_CACHE = {}


def kernel(features, wq, wk, wv, wo, q_norm_w, k_norm_w,
           num_heads=16, num_kv_heads=4, head_dim=64, sliding_window=512,
           **_unused):
    assert int(num_heads) == H and int(num_kv_heads) == KV
    assert int(head_dim) == D and int(sliding_window) == WIN
    from concourse.bass_utils import run_bass_kernel_spmd

    if "nc" not in _NC_CACHE:
        _NC_CACHE["nc"] = build_nc(for_sim=False)
    nc = _NC_CACHE["nc"]
    in_maps = make_in_maps(features, wq, wk, wv, wo, q_norm_w, k_norm_w)
    res = run_bass_kernel_spmd(nc, in_maps, core_ids=list(range(N_CORES)))
    outs = [r["out"] for r in res.results]
    full = np.stack(outs, axis=0).reshape(B, NCHUNK * SQ, M)
    return full.astype(np.float32)


# revision 33
# speedup vs baseline: 1.0148x; 1.0148x over previous
"""Trainium2 Bass kernel for CodecAttention (GQA + full-width RMSNorm + ALiBi
+ 512 sliding causal window), SPMD over 8 NeuronCores.

Sharding: 2 batches x 4 sequence chunks of 512 queries per core. Each core
sees a [q0-512, q0+512) feature slice (zero-padded left halo for chunk 0),
computes its own QKV projections + norms + banded windowed attention + output
projection. Host only slices/transposes/casts inputs and concatenates outputs.

v3 structure:
- bf16 operands for projections / PV / out-proj; f32r for attention scores
  (the in-matmul bias rows need >8 mantissa bits for exact key positions).
- ALiBi bias slope*(pos-512), the per-query exponent correction -slope*q and
  the chunk-0 halo kill are all folded into the score matmul via two spare
  contraction rows (K-side: pos-512 / ones; Q-side: 8*slope / -8*slope*q).
  The exp activation is then bias-free, so score tiles for one head are
  packed into double-buffered PSUM chunks (2+2+1 banks, zero garbage) and
  exponentiated with three large ACT calls per head.
- rmsnorm weights are folded into wq/wk host-side; all row broadcasts
  (rms reciprocals, softmax denominators) go through gpsimd
  partition_broadcast instead of PE matmuls.
- Projection staging copies are split between the scalar and vector engines;
  triangle masks alternate between gpsimd affine_select and DVE mask-muls
  so no single engine serializes the attention inner loop.
"""

import math

import numpy as np
import ml_dtypes

import concourse.bass as bass
import concourse.tile as tile
from concourse import bacc, mybir

F32 = mybir.dt.float32
F32R = mybir.dt.float32r
BF16 = mybir.dt.bfloat16
AF = mybir.ActivationFunctionType
NPBF16 = ml_dtypes.bfloat16

# Problem constants (hardcoded per spec nn_CodecAttention_34308198761010)
B, S, M = 2, 2048, 1024
H, KV, D = 16, 4, 64
WIN = 512
SQ = 512          # queries per core
SK = 1024         # k-range per core (halo + chunk)
NCHUNK = S // SQ  # 4
N_CORES = 8
EPS = 1e-6
# Head order permutation (host-side, applied to wq/q_norm_w/wo): Q-head slot
# 2j+sub sits at partition offset 64*sub, which must equal its KV head's
# partition offset 64*((h//4)%2) (matmul requires equal base partitions).
PERM = [0, 4, 1, 5, 2, 6, 3, 7, 8, 12, 9, 13, 10, 14, 11, 15]

# banded spans: for k-tile kt (128 rows of kl), valid q columns [QL[kt], QR[kt])
QL = [0, 0, 0, 0, 0, 128, 256, 384]
QR = [128, 256, 384, 512, 512, 512, 512, 512]

# Score-band PSUM chunks (each (kt, col offset); every matmul output inside
# one 512-float bank, zero garbage columns):
#   C2A = 2 banks {kt3, kt4}, C2B = 2 banks {kt2, kt0, kt5, kt7},
#   C1 = 1 bank {kt1, kt6}
CHUNK_C2A = [(3, 0), (4, 512)]
CHUNK_C2B = [(2, 0), (0, 384), (5, 512), (7, 896)]
CHUNK_C1 = [(1, 0), (6, 256)]
# diagonal-block masks: (col offset, type, engine) per chunk.
# type 1: keep col' <= p (window bound, kt<=3); type 2: keep col' >= p
# (causal bound, kt>=4). engine 0 = gpsimd affine_select, 1 = DVE mask-mul.
MASKS_C2A = [(384, 1, 0), (512, 2, 1)]
MASKS_C2B = [(256, 1, 0), (384, 1, 1), (512, 2, 0), (896, 2, 1)]
MASKS_C1 = [(128, 1, 0), (256, 2, 1)]


def _alibi_slopes(n):
    ratio = 2.0 ** (-8.0 / n)
    return np.array([ratio**i for i in range(n)], dtype=np.float64)


def build_nc(for_sim=False):
    nc = bacc.Bacc(None, target_bir_lowering=False, debug=for_sim)

    feat_d = nc.dram_tensor("feat", [128, 8, SK], BF16, kind="ExternalInput")
    wq_d = nc.dram_tensor("wq", [8, 128, 8, 128], BF16, kind="ExternalInput")
    wk_d = nc.dram_tensor("wk", [128, 8, 256], BF16, kind="ExternalInput")
    wv_d = nc.dram_tensor("wv", [128, 8, 256], BF16, kind="ExternalInput")
    wo_d = nc.dram_tensor("wo", [128, 8, M], BF16, kind="ExternalInput")
    ones_d = nc.dram_tensor("onesin", [128, 128], F32R, kind="ExternalInput")
    sel_d = nc.dram_tensor("sel", [128, 2, 128], F32R, kind="ExternalInput")
    tri_d = nc.dram_tensor("tri", [128, 2, 128], BF16, kind="ExternalInput")
    vone_d = nc.dram_tensor("vone", [128, 8, KV, 1], BF16,
                            kind="ExternalInput")
    kbias_d = nc.dram_tensor("kbias", [3, 2, SK], BF16, kind="ExternalInput")
    qbias_d = nc.dram_tensor("qbias", [2, 3, 8, SQ], BF16,
                             kind="ExternalInput")
    out_d = nc.dram_tensor("out", [SQ, M], F32, kind="ExternalOutput")

    with tile.TileContext(nc) as tc:
        with (
            nc.allow_low_precision("low precision matmul operands are "
                                   "intentional"),
            tc.tile_pool(name="const", bufs=1) as constp,
            tc.tile_pool(name="acts", bufs=1) as actsp,
            tc.tile_pool(name="se2a", bufs=2) as se2ap,
            tc.tile_pool(name="se2b", bufs=2) as se2bp,
            tc.tile_pool(name="se1", bufs=2) as se1p,
            tc.tile_pool(name="sqpool", bufs=2) as sqp,
            tc.tile_pool(name="small", bufs=1) as smallp,
            tc.tile_pool(name="outsb", bufs=2) as outp,
        ):
            # ---- shared constant tiles (DMAs issued after feat below) ----
            ones_sb = constp.tile([128, 128], F32R)
            sel_sb = constp.tile([128, 2, 128], F32R)
            tri_sb = constp.tile([128, 2, 128], BF16)
            eps_sb = constp.tile([128, 1], F32)
            nc.vector.memset(eps_sb[:], EPS)

            # long-lived activations.
            # kt_sb[:, par, gi, :]: kv head g=2*gi+par; data rows 64*par..+64,
            # bias rows (pos-512, ones) in the opposite half, rest zero.
            # qt_sb[:, par, i, :]: head PERM[2*i+par]; same row scheme with
            # Q-side bias rows (8*slope, -8*slope*q).
            kt_sb = actsp.tile([128, 2, 2, SK], BF16)
            qt_sb = actsp.tile([128, 2, 8, SQ], BF16)
            # v_sb cols: [0:64 V][64 ones]: PV rows 0:64, denom row 64
            v_sb = actsp.tile([128, 8, KV, 65], BF16)
            attn_sb = actsp.tile([128, 8, SQ], BF16)

            # ================= projection phase (scoped SBUF) ==========
            with (
                tc.tile_pool(name="wkv", bufs=1) as wkvp,
                tc.tile_pool(name="feat", bufs=1) as featp,
                tc.tile_pool(name="wstream", bufs=2) as wsp,
            ):
                wk_sb = wkvp.tile([128, 8, 256], BF16)
                wv_sb = wkvp.tile([128, 8, 256], BF16)
                feat_sb = featp.tile([128, 8, SK], BF16)
                nc.sync.dma_start(wk_sb[:], wk_d[:])
                for mt in range(8):
                    nc.sync.dma_start(feat_sb[:, mt, :], feat_d[:, mt, :])
                nc.sync.dma_start(wv_sb[:], wv_d[:])
                nc.sync.dma_start(ones_sb[:], ones_d[:])
                nc.sync.dma_start(sel_sb[:], sel_d[:])
                nc.sync.dma_start(tri_sb[:], tri_d[:])
                nc.sync.dma_start(v_sb[:, :, :, 64:65], vone_d[:])
                # zero-init tiles whose unwritten rows feed sel-broadcast
                # matmuls (only one row carries data; the sel matrix zeroes
                # the rest, but they must hold finite values)
                srtk = smallp.tile([128, SK], F32R, tag="srtk", name="srtk")
                srtq = smallp.tile([128, SQ], F32R, tag="srtq", name="srtq")
                den0 = smallp.tile([128, 2, SQ], F32R, tag="den", name="den")
                nc.vector.tensor_scalar_mul(srtk[:], feat_sb[:, 0, :], 0.0)
                nc.vector.tensor_scalar_mul(srtq[:], feat_sb[:, 0, 0:512],
                                            0.0)
                nc.vector.tensor_scalar_mul(
                    den0[:].rearrange("p a b -> p (a b)"),
                    feat_sb[:, 1, :], 0.0)
                # zero the spare halves of kt/qt once while DMAs stream in
                # (src operand is only for shape; value is ignored for *0.0)
                nc.vector.tensor_scalar_mul(
                    kt_sb[64:128, 0, :, :].rearrange("p a b -> p (a b)"),
                    feat_sb[64:128, 0:2, :].rearrange("p a b -> p (a b)"),
                    0.0)
                nc.vector.tensor_scalar_mul(
                    kt_sb[0:64, 1, :, :].rearrange("p a b -> p (a b)"),
                    feat_sb[0:64, 0:2, :].rearrange("p a b -> p (a b)"),
                    0.0)
                nc.vector.tensor_scalar_mul(
                    qt_sb[64:128, 0, :, :].rearrange("p a b -> p (a b)"),
                    feat_sb[64:128, 0:4, :].rearrange("p a b -> p (a b)"),
                    0.0)
                nc.vector.tensor_scalar_mul(
                    qt_sb[0:64, 1, :, :].rearrange("p a b -> p (a b)"),
                    feat_sb[0:64, 0:4, :].rearrange("p a b -> p (a b)"),
                    0.0)

                # ---- K/V/Q projections: V-pool outer so V matmuls fill
                # the gaps in K's rmsnorm tail; ssq/broadcast tiles rotate
                # through one small pool to stay inside 8 PSUM banks ----
                with tc.tile_pool(name="psV", bufs=2,
                                  space=bass.MemorySpace.PSUM) as psV:
                    with (
                        tc.tile_pool(name="psK", bufs=1,
                                     space=bass.MemorySpace.PSUM) as psK,
                        tc.tile_pool(name="psKN", bufs=1,
                                     space=bass.MemorySpace.PSUM) as psKN,
                    ):
                        kp = [psK.tile([128, SK], F32, tag=f"kp{t}",
                                       name=f"kp{t}")
                              for t in range(2)]
                        # mt-major so matmuls start as soon as each feature
                        # slice lands
                        for mt in range(8):
                            for t in range(2):
                                for half in range(2):
                                    cs = slice(512 * half, 512 * half + 512)
                                    nc.tensor.matmul(
                                        kp[t][:, cs],
                                        wk_sb[:, mt, 128 * t:128 * t + 128],
                                        feat_sb[:, mt, cs],
                                        start=(mt == 0), stop=(mt == 7))
                        ssqk = psKN.tile([128, SK], F32, tag="kn",
                                         name="ssqk")
                        for t in range(2):
                            sqk = sqp.tile([128, SK], F32R, tag="sqk")
                            nc.scalar.activation(sqk[:], kp[t][:], AF.Square)
                            for half in range(2):
                                cs = slice(512 * half, 512 * half + 512)
                                nc.tensor.matmul(ssqk[0:1, cs],
                                                 ones_sb[:, 0:1],
                                                 sqk[:, cs],
                                                 start=(t == 0),
                                                 stop=(t == 1))
                        nc.scalar.activation(srtk[0:1, :], ssqk[0:1, :],
                                             AF.Sqrt, scale=1.0 / 256.0,
                                             bias=eps_sb[0:1, :])
                        bck = psKN.tile([128, SK], F32, tag="kn",
                                        name="bck")
                        for half in range(2):
                            cs = slice(512 * half, 512 * half + 512)
                            nc.tensor.matmul(bck[:, cs], sel_sb[:, 0, :],
                                             srtk[:, cs], start=True,
                                             stop=True)
                        bcki = smallp.tile([128, SK], F32, tag="bcki")
                        nc.vector.reciprocal_approx_fast(bcki[:], bck[:])
                        # fused stage+norm: kt = kp * (1/rms) from PSUM
                        for t in range(2):
                            nc.vector.tensor_mul(kt_sb[0:64, 0, t, :],
                                                 kp[t][0:64, :],
                                                 bcki[0:64, :])
                            nc.vector.tensor_mul(kt_sb[64:128, 1, t, :],
                                                 kp[t][64:128, :],
                                                 bcki[64:128, :])

                        # ---- V projection (fills K norm-chain gaps) ----
                        for st in range(8):
                            vp = psV.tile([128, 256], F32, tag="vp",
                                          name="vp")
                            for mt in range(8):
                                nc.tensor.matmul(
                                    vp[:],
                                    feat_sb[:, mt, 128 * st:128 * st + 128],
                                    wv_sb[:, mt, :],
                                    start=(mt == 0), stop=(mt == 7))
                            nc.vector.tensor_copy(
                                v_sb[:, st, :, 0:64],
                                vp[:].rearrange("p (g d) -> p g d", g=KV))

                    # ---- Q projection + rmsnorm ----
                    with (
                        tc.tile_pool(name="psQ", bufs=2,
                                     space=bass.MemorySpace.PSUM) as psQ,
                        tc.tile_pool(name="psQN", bufs=1,
                                     space=bass.MemorySpace.PSUM) as psQN,
                    ):
                        ssqq = psQN.tile([128, SQ], F32, tag="qn",
                                         name="ssqq")
                        for t in range(8):
                            wqt = wsp.tile([128, 8, 128], BF16, tag="wqt")
                            nc.sync.dma_start(wqt[:], wq_d[t])
                            qp = psQ.tile([128, SQ], F32)
                            for mt in range(8):
                                nc.tensor.matmul(qp[:], wqt[:, mt, :],
                                                 feat_sb[:, mt, 512:1024],
                                                 start=(mt == 0),
                                                 stop=(mt == 7))
                            sqq = sqp.tile([128, SQ], F32R, tag="sqq")
                            nc.scalar.activation(sqq[:], qp[:], AF.Square)
                            nc.tensor.matmul(ssqq[0:1, :], ones_sb[:, 0:1],
                                             sqq[:],
                                             start=(t == 0), stop=(t == 7))
                            # staging copies split across scalar/vector
                            nc.scalar.copy(qt_sb[0:64, 0, t, :], qp[0:64, :])
                            nc.vector.tensor_copy(qt_sb[64:128, 1, t, :],
                                                  qp[64:128, :])
                        nc.scalar.activation(srtq[0:1, :], ssqq[0:1, :],
                                             AF.Sqrt, scale=1.0 / 1024.0,
                                             bias=eps_sb[0:1, :])
                        bcq = psQN.tile([128, SQ], F32, tag="qn",
                                        name="bcq")
                        nc.tensor.matmul(bcq[:], sel_sb[:, 0, :],
                                         srtq[:, :], start=True, stop=True)
                        bcqi = smallp.tile([128, SQ], F32, tag="bcqi")
                        nc.vector.reciprocal_approx_fast(bcqi[:], bcq[:])
                        for t in range(8):
                            nc.vector.tensor_mul(qt_sb[0:64, 0, t, :],
                                                 qt_sb[0:64, 0, t, :],
                                                 bcqi[0:64, :])
                            nc.vector.tensor_mul(qt_sb[64:128, 1, t, :],
                                                 qt_sb[64:128, 1, t, :],
                                                 bcqi[64:128, :])

            # ================= attention phase ========================
            # bias rows overwrite two rows of the zeroed spare halves
            nc.sync.dma_start(kt_sb[64:67, 0, :, :], kbias_d[:])
            nc.sync.dma_start(kt_sb[0:3, 1, :, :], kbias_d[:])
            nc.sync.dma_start(qt_sb[64:67, 0, :, :], qbias_d[0])
            nc.sync.dma_start(qt_sb[0:3, 1, :, :], qbias_d[1])
            with (
                tc.tile_pool(name="wbig", bufs=1) as wbigp,
            ):
                wo_sb = wbigp.tile([128, 8, M], BF16)
                for c in range(2):
                    nc.sync.dma_start(wo_sb[:, 4 * c:4 * c + 4, :],
                                      wo_d[:, 4 * c:4 * c + 4, :])

                with (
                    tc.tile_pool(name="psC2", bufs=2,
                                 space=bass.MemorySpace.PSUM) as psC2,
                    tc.tile_pool(name="psC1", bufs=1,
                                 space=bass.MemorySpace.PSUM) as psC1,
                    tc.tile_pool(name="psPV", bufs=1,
                                 space=bass.MemorySpace.PSUM) as psPV,
                    tc.tile_pool(name="psBC", bufs=1,
                                 space=bass.MemorySpace.PSUM) as psBC,
                ):
                    pv_pair = [None, None]
                    for s in range(16):
                        par, idx = s % 2, s // 2
                        h = PERM[s]
                        g = h // 4
                        assert g % 2 == par
                        gi = g // 2
                        pv = psPV.tile([128, SQ], F32, tag=f"pv{s % 2}",
                                       name=f"pv{s % 2}")
                        pv_pair[par] = pv

                        ps2a = psC2.tile([128, 1024], F32, tag="c2",
                                         name="ps2a")
                        ps2b = psC2.tile([128, 1024], F32, tag="c2",
                                         name="ps2b")
                        ps1 = psC1.tile([128, 512], F32, tag="c1",
                                        name="ps1")
                        chunks = ((CHUNK_C2A, ps2a), (CHUNK_C2B, ps2b),
                                  (CHUNK_C1, ps1))
                        for chunk, ps in chunks:
                            for kt, off in chunk:
                                span = QR[kt] - QL[kt]
                                nc.tensor.matmul(
                                    ps[:, off:off + span],
                                    kt_sb[:, par, gi,
                                          128 * kt:128 * kt + 128],
                                    qt_sb[:, par, idx, QL[kt]:QR[kt]],
                                    start=True, stop=True)
                        se2a = se2ap.tile([128, 1024], BF16, tag="se2a")
                        se2b = se2bp.tile([128, 1024], BF16, tag="se2b")
                        se1 = se1p.tile([128, 512], BF16, tag="se1")
                        semap = ((se2a, ps2a, MASKS_C2A),
                                 (se2b, ps2b, MASKS_C2B),
                                 (se1, ps1, MASKS_C1))
                        for se, ps, masks in semap:
                            nc.scalar.activation(se[:], ps[:], AF.Exp,
                                                 scale=0.125)
                            for off, mtype, eng in masks:
                                blk = se[:, off:off + 128]
                                if eng == 0:
                                    if mtype == 1:  # keep col' <= p
                                        nc.gpsimd.affine_select(
                                            blk, blk, pattern=[[-1, 128]],
                                            compare_op=mybir.AluOpType.is_ge,
                                            fill=0.0, base=0,
                                            channel_multiplier=1)
                                    else:  # keep col' >= p
                                        nc.gpsimd.affine_select(
                                            blk, blk, pattern=[[1, 128]],
                                            compare_op=mybir.AluOpType.is_ge,
                                            fill=0.0, base=0,
                                            channel_multiplier=-1)
                                else:
                                    nc.vector.tensor_mul(
                                        blk, blk,
                                        tri_sb[:, mtype - 1, :])
                        nmm = 0
                        sechunks = ((CHUNK_C2A, se2a), (CHUNK_C2B, se2b),
                                    (CHUNK_C1, se1))
                        for chunk, se in sechunks:
                            for kt, off in chunk:
                                span = QR[kt] - QL[kt]
                                nc.tensor.matmul(
                                    pv[0:65, QL[kt]:QR[kt]],
                                    v_sb[:, kt, g, 0:65],
                                    se[:, off:off + span],
                                    start=(nmm == 0), stop=(nmm == 7))
                                nmm += 1

                        if par == 1:
                            # finalize pair: attn = pv / denom per head
                            # (reciprocal runs after the broadcast on a
                            # [64, SQ] tile -- single-partition custom-DVE
                            # reciprocals misbehave on hardware)
                            p_i = idx
                            den = den0
                            nc.vector.tensor_copy(den[64:65, 0, :],
                                                  pv_pair[0][64:65, :])
                            nc.vector.tensor_copy(den[64:65, 1, :],
                                                  pv_pair[1][64:65, :])
                            bc0 = psBC.tile([64, SQ], F32, tag="bc",
                                            name="bc0")
                            nc.tensor.matmul(bc0[:], sel_sb[:, 1, 0:64],
                                             den[:, 0, :],
                                             start=True, stop=True)
                            bci0 = smallp.tile([64, SQ], F32, tag="bci0")
                            nc.vector.reciprocal_approx_fast(bci0[:],
                                                             bc0[:])
                            nc.vector.tensor_mul(attn_sb[0:64, p_i, :],
                                                 pv_pair[0][0:64, :],
                                                 bci0[:])
                            bc1 = psBC.tile([64, SQ], F32, tag="bc",
                                            name="bc1")
                            nc.tensor.matmul(bc1[:], sel_sb[:, 1, 0:64],
                                             den[:, 1, :],
                                             start=True, stop=True)
                            bci1 = smallp.tile([64, SQ], F32, tag="bci1")
                            nc.vector.reciprocal_approx_fast(bci1[:],
                                                             bc1[:])
                            atmp = smallp.tile([64, SQ], BF16, tag="atmp")
                            nc.vector.tensor_mul(atmp[:],
                                                 pv_pair[1][0:64, :],
                                                 bci1[:])
                            nc.sync.dma_start(attn_sb[64:128, p_i, :],
                                              atmp[:])

                # ---- output projection (wo still resident) ----
                outv = out_d.rearrange("(st p) m -> st p m", p=128)
                with tc.tile_pool(name="psO", bufs=4,
                                  space=bass.MemorySpace.PSUM) as psO:
                    for st in range(4):
                        osb = outp.tile([128, M], F32, tag="osb")
                        for mh in range(2):
                            op = psO.tile([128, 512], F32)
                            for ht in range(8):
                                nc.tensor.matmul(
                                    op[:],
                                    attn_sb[:, ht, 128 * st:128 * st + 128],
                                    wo_sb[:, ht, 512 * mh:512 * mh + 512],
                                    start=(ht == 0), stop=(ht == 7))
                            if mh == 0:
                                nc.scalar.copy(
                                    osb[:, 512 * mh:512 * mh + 512], op[:])
                            else:
                                nc.vector.tensor_copy(
                                    osb[:, 512 * mh:512 * mh + 512], op[:])
                        nc.sync.dma_start(outv[st], osb[:])

    if for_sim:
        nc.compile()
    else:
        nc.finalize()
    return nc


def make_in_maps(features, wq, wk, wv, wo, q_norm_w, k_norm_w):
    features = np.asarray(features, np.float32)
    wq = np.asarray(wq, np.float32)
    wk = np.asarray(wk, np.float32)
    wv = np.asarray(wv, np.float32)
    wo = np.asarray(wo, np.float32)
    q_norm_w = np.asarray(q_norm_w, np.float32)
    k_norm_w = np.asarray(k_norm_w, np.float32)

    # permute Q-head order (see PERM) in wq rows, q_norm_w, wo columns;
    # fold the rmsnorm weights into the projection rows (commutes with the
    # per-position rms scaling)
    wq_p = wq.reshape(H, D, M)[PERM].reshape(H * D, M)
    qnw_p = q_norm_w.reshape(H, D)[PERM].reshape(H * D)
    wq_p = wq_p * qnw_p[:, None]
    wk_f = wk * k_norm_w[:, None]
    wo_tp = wo.T.reshape(H, D, M)[PERM].reshape(H * D, M)  # wo.T rows = hd

    wq_pre = np.ascontiguousarray(
        wq_p.T.reshape(8, 128, 8, 128).transpose(2, 1, 0, 3)).astype(NPBF16)
    wk_pre = np.ascontiguousarray(
        wk_f.T.reshape(8, 128, 256).transpose(1, 0, 2)).astype(NPBF16)
    wv_pre = np.ascontiguousarray(
        wv.T.reshape(8, 128, 256).transpose(1, 0, 2)).astype(NPBF16)
    wo_pre = np.ascontiguousarray(
        wo_tp.reshape(8, 128, M).transpose(1, 0, 2)).astype(NPBF16)

    slopes = _alibi_slopes(H)

    vone = np.ones((128, 8, KV, 1), NPBF16)
    # row selectors: sel[:,0,:] picks partition 0, sel[:,1,:] picks 64
    sel = np.zeros((128, 2, 128), np.float32)
    sel[0, 0, :] = 1.0
    sel[64, 1, :] = 1.0
    # triangle mask tiles for the DVE mask-muls: tri[:,0]=keep col<=p,
    # tri[:,1]=keep col>=p
    p = np.arange(128)
    tri = np.zeros((128, 2, 128), NPBF16)
    tri[:, 0, :] = (p[None, :] <= p[:, None])
    tri[:, 1, :] = (p[None, :] >= p[:, None])

    # Q-side bias rows: rows 0/1 = 8*slope_h (pair with K-side pos_hi and
    # pos_lo; the position is split so both parts are bf16-exact),
    # row 2 = -8*slope_h*q - 320 (pairs with K-side ones; cancels in
    # softmax, keeps exp args <= ~-28 valid / < ~65 in masked triangles so
    # nothing overflows to inf before the mask-muls)
    qi = np.arange(SQ, dtype=np.float64)
    qbias = np.zeros((2, 3, 8, SQ), np.float32)
    for s in range(16):
        par, idx = s % 2, s // 2
        sl = slopes[PERM[s]]
        qbias[par, 0, idx, :] = 8.0 * sl
        qbias[par, 1, idx, :] = 8.0 * sl
        qbias[par, 2, idx, :] = -8.0 * sl * qi - 320.0
    qbias = qbias.astype(NPBF16)

    in_maps = []
    for b in range(B):
        for c in range(NCHUNK):
            q0 = c * SQ
            lo, hi = q0 - WIN, q0 + SQ
            fs = np.zeros((SK, M), np.float32)
            src_lo = max(lo, 0)
            fs[src_lo - lo:, :] = features[b, src_lo:hi, :]
            feat_pre = np.ascontiguousarray(
                fs.T.reshape(8, 128, SK).transpose(1, 0, 2)).astype(NPBF16)
            # K-side bias rows: rows 0/1 = pos_hi/pos_lo with
            # pos_hi + pos_lo = pos-512, both bf16-exact (halo positions
            # get a huge negative value so exp underflows to 0); row 2 = 1
            kbias = np.zeros((3, 2, SK), np.float32)
            pos = np.arange(SK, dtype=np.float64) - 512.0
            pos_hi = 4.0 * np.floor(pos / 4.0)
            pos_lo = pos - pos_hi
            if c == 0:
                pos_hi[:512] = -1e30
                pos_lo[:512] = 0.0
            kbias[0, :, :] = pos_hi[None, :]
            kbias[1, :, :] = pos_lo[None, :]
            kbias[2, :, :] = 1.0
            kbias = kbias.astype(NPBF16)
            in_maps.append({
                "feat": feat_pre, "wq": wq_pre, "wk": wk_pre, "wv": wv_pre,
                "wo": wo_pre,
                "onesin": np.ones((128, 128), np.float32),
                "sel": sel, "tri": tri, "vone": vone,
                "kbias": kbias, "qbias": qbias,
            })
    return in_maps


_NC_CACHE = {}


def kernel(features, wq, wk, wv, wo, q_norm_w, k_norm_w,
           num_heads=16, num_kv_heads=4, head_dim=64, sliding_window=512,
           **_unused):
    assert int(num_heads) == H and int(num_kv_heads) == KV
    assert int(head_dim) == D and int(sliding_window) == WIN
    from concourse.bass_utils import run_bass_kernel_spmd

    if "nc" not in _NC_CACHE:
        _NC_CACHE["nc"] = build_nc(for_sim=False)
    nc = _NC_CACHE["nc"]
    in_maps = make_in_maps(features, wq, wk, wv, wo, q_norm_w, k_norm_w)
    res = run_bass_kernel_spmd(nc, in_maps, core_ids=list(range(N_CORES)))
    outs = [r["out"] for r in res.results]
    full = np.stack(outs, axis=0).reshape(B, NCHUNK * SQ, M)
    return full.astype(np.float32)


# revision 34
# speedup vs baseline: 1.0876x; 1.0717x over previous
"""Trainium2 Bass kernel for CodecAttention (GQA + full-width RMSNorm + ALiBi
+ 512 sliding causal window), SPMD over 8 NeuronCores.

Sharding: 2 batches x 4 sequence chunks of 512 queries per core. Each core
sees a [q0-512, q0+512) feature slice (zero-padded left halo for chunk 0),
computes its own QKV projections + norms + banded windowed attention + output
projection. Host only slices/transposes/casts inputs and concatenates outputs.

v3 structure:
- bf16 operands for projections / PV / out-proj; f32r for attention scores
  (the in-matmul bias rows need >8 mantissa bits for exact key positions).
- ALiBi bias slope*(pos-512), the per-query exponent correction -slope*q and
  the chunk-0 halo kill are all folded into the score matmul via two spare
  contraction rows (K-side: pos-512 / ones; Q-side: 8*slope / -8*slope*q).
  The exp activation is then bias-free, so score tiles for one head are
  packed into double-buffered PSUM chunks (2+2+1 banks, zero garbage) and
  exponentiated with three large ACT calls per head.
- rmsnorm weights are folded into wq/wk host-side; all row broadcasts
  (rms reciprocals, softmax denominators) go through gpsimd
  partition_broadcast instead of PE matmuls.
- Projection staging copies are split between the scalar and vector engines;
  triangle masks alternate between gpsimd affine_select and DVE mask-muls
  so no single engine serializes the attention inner loop.
"""

import math

import numpy as np
import ml_dtypes

import concourse.bass as bass
import concourse.tile as tile
from concourse import bacc, mybir

F32 = mybir.dt.float32
F32R = mybir.dt.float32r
BF16 = mybir.dt.bfloat16
AF = mybir.ActivationFunctionType
NPBF16 = ml_dtypes.bfloat16

# Problem constants (hardcoded per spec nn_CodecAttention_34308198761010)
B, S, M = 2, 2048, 1024
H, KV, D = 16, 4, 64
WIN = 512
SQ = 512          # queries per core
SK = 1024         # k-range per core (halo + chunk)
NCHUNK = S // SQ  # 4
N_CORES = 8
EPS = 1e-6
# Head order permutation (host-side, applied to wq/q_norm_w/wo): Q-head slot
# 2j+sub sits at partition offset 64*sub, which must equal its KV head's
# partition offset 64*((h//4)%2) (matmul requires equal base partitions).
PERM = [0, 4, 1, 5, 2, 6, 3, 7, 8, 12, 9, 13, 10, 14, 11, 15]

# banded spans: for k-tile kt (128 rows of kl), valid q columns [QL[kt], QR[kt])
QL = [0, 0, 0, 0, 0, 128, 256, 384]
QR = [128, 256, 384, 512, 512, 512, 512, 512]

# Score-band PSUM chunks (each (kt, col offset); every matmul output inside
# one 512-float bank, zero garbage columns):
#   C2A = 2 banks {kt3, kt4}, C2B = 2 banks {kt2, kt0, kt5, kt7},
#   C1 = 1 bank {kt1, kt6}
CHUNK_C2A = [(3, 0), (4, 512)]
CHUNK_C2B = [(2, 0), (0, 384), (5, 512), (7, 896)]
CHUNK_C1 = [(1, 0), (6, 256)]
# diagonal-block masks: (col offset, type, engine) per chunk.
# type 1: keep col' <= p (window bound, kt<=3); type 2: keep col' >= p
# (causal bound, kt>=4). engine 0 = gpsimd affine_select, 1 = DVE mask-mul.
MASKS_C2A = [(384, 1, 0), (512, 2, 1)]
MASKS_C2B = [(256, 1, 0), (384, 1, 1), (512, 2, 0), (896, 2, 1)]
MASKS_C1 = [(128, 1, 0), (256, 2, 1)]


def _alibi_slopes(n):
    ratio = 2.0 ** (-8.0 / n)
    return np.array([ratio**i for i in range(n)], dtype=np.float64)


def build_nc(for_sim=False):
    nc = bacc.Bacc(None, target_bir_lowering=False, debug=for_sim)

    feat_d = nc.dram_tensor("feat", [128, 8, SK], BF16, kind="ExternalInput")
    wq_d = nc.dram_tensor("wq", [8, 128, 8, 128], BF16, kind="ExternalInput")
    wk_d = nc.dram_tensor("wk", [128, 8, 256], BF16, kind="ExternalInput")
    wv_d = nc.dram_tensor("wv", [128, 8, 256], BF16, kind="ExternalInput")
    wo_d = nc.dram_tensor("wo", [128, 8, M], BF16, kind="ExternalInput")
    ones_d = nc.dram_tensor("onesin", [128, 128], F32R, kind="ExternalInput")
    sel_d = nc.dram_tensor("sel", [128, 2, 128], F32R, kind="ExternalInput")
    tri_d = nc.dram_tensor("tri", [128, 2, 128], BF16, kind="ExternalInput")
    vone_d = nc.dram_tensor("vone", [128, 8, KV, 1], BF16,
                            kind="ExternalInput")
    kbias_d = nc.dram_tensor("kbias", [3, 2, SK], BF16, kind="ExternalInput")
    qbias_d = nc.dram_tensor("qbias", [2, 3, 8, SQ], BF16,
                             kind="ExternalInput")
    out_d = nc.dram_tensor("out", [SQ, M], F32, kind="ExternalOutput")

    with tile.TileContext(nc) as tc:
        with (
            nc.allow_low_precision("low precision matmul operands are "
                                   "intentional"),
            tc.tile_pool(name="const", bufs=1) as constp,
            tc.tile_pool(name="acts", bufs=1) as actsp,
            tc.tile_pool(name="se2a", bufs=2) as se2ap,
            tc.tile_pool(name="se2b", bufs=2) as se2bp,
            tc.tile_pool(name="se1", bufs=2) as se1p,
            tc.tile_pool(name="sqpool", bufs=2) as sqp,
            tc.tile_pool(name="small", bufs=1) as smallp,
            tc.tile_pool(name="outsb", bufs=2) as outp,
        ):
            # ---- shared constant tiles (DMAs issued after feat below) ----
            ones_sb = constp.tile([128, 128], F32R)
            sel_sb = constp.tile([128, 2, 128], F32R)
            tri_sb = constp.tile([128, 2, 128], BF16)
            eps_sb = constp.tile([128, 1], F32)
            nc.vector.memset(eps_sb[:], EPS)

            # long-lived activations.
            # kt_sb[:, par, gi, :]: kv head g=2*gi+par; data rows 64*par..+64,
            # bias rows (pos-512, ones) in the opposite half, rest zero.
            # qt_sb[:, par, i, :]: head PERM[2*i+par]; same row scheme with
            # Q-side bias rows (8*slope, -8*slope*q).
            kt_sb = actsp.tile([128, 2, 2, SK], BF16)
            qt_sb = actsp.tile([128, 2, 8, SQ], BF16)
            # v_sb cols: [0:64 V][64 ones]: PV rows 0:64, denom row 64
            v_sb = actsp.tile([128, 8, KV, 65], BF16)
            attn_sb = actsp.tile([128, 8, SQ], BF16)

            # ================= projection phase (scoped SBUF) ==========
            with (
                tc.tile_pool(name="wkv", bufs=1) as wkvp,
                tc.tile_pool(name="feat", bufs=1) as featp,
                tc.tile_pool(name="wstream", bufs=2) as wsp,
            ):
                wk_sb = wkvp.tile([128, 8, 256], BF16)
                wv_sb = wkvp.tile([128, 8, 256], BF16)
                feat_sb = featp.tile([128, 8, SK], BF16)
                nc.sync.dma_start(wk_sb[:], wk_d[:])
                for mt in range(8):
                    nc.sync.dma_start(feat_sb[:, mt, :], feat_d[:, mt, :])
                nc.sync.dma_start(wv_sb[:], wv_d[:])
                nc.sync.dma_start(ones_sb[:], ones_d[:])
                nc.sync.dma_start(sel_sb[:], sel_d[:])
                nc.sync.dma_start(tri_sb[:], tri_d[:])
                nc.sync.dma_start(v_sb[:, :, :, 64:65], vone_d[:])
                # zero-init tiles whose unwritten rows feed sel-broadcast
                # matmuls (only one row carries data; the sel matrix zeroes
                # the rest, but they must hold finite values)
                srtk = smallp.tile([128, SK], F32R, tag="srtk", name="srtk")
                srtq = smallp.tile([128, SQ], F32R, tag="srtq", name="srtq")
                den0 = smallp.tile([128, 2, SQ], F32R, tag="den", name="den")
                nc.vector.tensor_scalar_mul(srtk[:], feat_sb[:, 0, :], 0.0)
                nc.vector.tensor_scalar_mul(srtq[:], feat_sb[:, 0, 0:512],
                                            0.0)
                nc.vector.tensor_scalar_mul(
                    den0[:].rearrange("p a b -> p (a b)"),
                    feat_sb[:, 1, :], 0.0)
                # zero the spare halves of kt/qt once while DMAs stream in
                # (src operand is only for shape; value is ignored for *0.0)
                nc.vector.tensor_scalar_mul(
                    kt_sb[64:128, 0, :, :].rearrange("p a b -> p (a b)"),
                    feat_sb[64:128, 0:2, :].rearrange("p a b -> p (a b)"),
                    0.0)
                nc.vector.tensor_scalar_mul(
                    kt_sb[0:64, 1, :, :].rearrange("p a b -> p (a b)"),
                    feat_sb[0:64, 0:2, :].rearrange("p a b -> p (a b)"),
                    0.0)
                nc.vector.tensor_scalar_mul(
                    qt_sb[64:128, 0, :, :].rearrange("p a b -> p (a b)"),
                    feat_sb[64:128, 0:4, :].rearrange("p a b -> p (a b)"),
                    0.0)
                nc.vector.tensor_scalar_mul(
                    qt_sb[0:64, 1, :, :].rearrange("p a b -> p (a b)"),
                    feat_sb[0:64, 0:4, :].rearrange("p a b -> p (a b)"),
                    0.0)

                # ---- K/V/Q projections in one PSUM ledger: K runs in two
                # 512-position halves (2 banks + 1 rotating norm bank) so
                # the Q and V pools coexist and fill K's rmsnorm tail ----
                with (
                    tc.tile_pool(name="psV", bufs=2,
                                 space=bass.MemorySpace.PSUM) as psV,
                    tc.tile_pool(name="psK", bufs=1,
                                 space=bass.MemorySpace.PSUM) as psK,
                    tc.tile_pool(name="psKN", bufs=1,
                                 space=bass.MemorySpace.PSUM) as psKN,
                    tc.tile_pool(name="psQ", bufs=2,
                                 space=bass.MemorySpace.PSUM) as psQ,
                    tc.tile_pool(name="psQN", bufs=1,
                                 space=bass.MemorySpace.PSUM) as psQN,
                ):
                    bcki = smallp.tile([128, SK], F32, tag="bcki",
                                       name="bcki")
                    for half in range(2):
                        cs = slice(512 * half, 512 * half + 512)
                        kp = [psK.tile([128, 512], F32, tag=f"kp{t}",
                                       name=f"kp{t}")
                              for t in range(2)]
                        # mt-major so matmuls start as each feature slice
                        # lands
                        for mt in range(8):
                            for t in range(2):
                                nc.tensor.matmul(
                                    kp[t][:],
                                    wk_sb[:, mt, 128 * t:128 * t + 128],
                                    feat_sb[:, mt, cs],
                                    start=(mt == 0), stop=(mt == 7))
                        ssqk = psKN.tile([128, 512], F32, tag="kn",
                                         name="ssqk")
                        for t in range(2):
                            sqk = sqp.tile([128, 512], F32R, tag="sqk")
                            nc.scalar.activation(sqk[:], kp[t][:], AF.Square)
                            nc.tensor.matmul(ssqk[0:1, :], ones_sb[:, 0:1],
                                             sqk[:],
                                             start=(t == 0), stop=(t == 1))
                        nc.scalar.activation(srtk[0:1, cs], ssqk[0:1, :],
                                             AF.Sqrt, scale=1.0 / 256.0,
                                             bias=eps_sb[0:1, :])
                        bck = psKN.tile([128, 512], F32, tag="kn",
                                        name="bck")
                        nc.tensor.matmul(bck[:], sel_sb[:, 0, :],
                                         srtk[:, cs], start=True, stop=True)
                        nc.vector.reciprocal_approx_fast(bcki[:, cs],
                                                         bck[:])
                        # fused stage+norm: kt = kp * (1/rms) from PSUM
                        for t in range(2):
                            nc.vector.tensor_mul(kt_sb[0:64, 0, t, cs],
                                                 kp[t][0:64, :],
                                                 bcki[0:64, cs])
                            nc.vector.tensor_mul(kt_sb[64:128, 1, t, cs],
                                                 kp[t][64:128, :],
                                                 bcki[64:128, cs])

                    # ---- Q projection + rmsnorm (overlaps K tail) ----
                    ssqq = psQN.tile([128, SQ], F32, tag="qn", name="ssqq")
                    for t in range(8):
                        wqt = wsp.tile([128, 8, 128], BF16, tag="wqt")
                        nc.sync.dma_start(wqt[:], wq_d[t])
                        qp = psQ.tile([128, SQ], F32)
                        for mt in range(8):
                            nc.tensor.matmul(qp[:], wqt[:, mt, :],
                                             feat_sb[:, mt, 512:1024],
                                             start=(mt == 0),
                                             stop=(mt == 7))
                        sqq = sqp.tile([128, SQ], F32R, tag="sqq")
                        nc.scalar.activation(sqq[:], qp[:], AF.Square)
                        nc.tensor.matmul(ssqq[0:1, :], ones_sb[:, 0:1],
                                         sqq[:],
                                         start=(t == 0), stop=(t == 7))
                        # staging copies split across scalar/vector
                        nc.scalar.copy(qt_sb[0:64, 0, t, :], qp[0:64, :])
                        nc.vector.tensor_copy(qt_sb[64:128, 1, t, :],
                                              qp[64:128, :])
                    nc.scalar.activation(srtq[0:1, :], ssqq[0:1, :],
                                         AF.Sqrt, scale=1.0 / 1024.0,
                                         bias=eps_sb[0:1, :])
                    bcq = psQN.tile([128, SQ], F32, tag="qn", name="bcq")
                    nc.tensor.matmul(bcq[:], sel_sb[:, 0, :],
                                     srtq[:, :], start=True, stop=True)
                    bcqi = smallp.tile([128, SQ], F32, tag="bcqi")
                    nc.vector.reciprocal_approx_fast(bcqi[:], bcq[:])
                    for t in range(8):
                        nc.vector.tensor_mul(qt_sb[0:64, 0, t, :],
                                             qt_sb[0:64, 0, t, :],
                                             bcqi[0:64, :])
                        nc.vector.tensor_mul(qt_sb[64:128, 1, t, :],
                                             qt_sb[64:128, 1, t, :],
                                             bcqi[64:128, :])

                    # ---- V projection (lowest priority, fills gaps) ----
                    for st in range(8):
                        vp = psV.tile([128, 256], F32, tag="vp", name="vp")
                        for mt in range(8):
                            nc.tensor.matmul(
                                vp[:],
                                feat_sb[:, mt, 128 * st:128 * st + 128],
                                wv_sb[:, mt, :],
                                start=(mt == 0), stop=(mt == 7))
                        nc.vector.tensor_copy(
                            v_sb[:, st, :, 0:64],
                            vp[:].rearrange("p (g d) -> p g d", g=KV))

            # ================= attention phase ========================
            # bias rows overwrite two rows of the zeroed spare halves
            nc.sync.dma_start(kt_sb[64:67, 0, :, :], kbias_d[:])
            nc.sync.dma_start(kt_sb[0:3, 1, :, :], kbias_d[:])
            nc.sync.dma_start(qt_sb[64:67, 0, :, :], qbias_d[0])
            nc.sync.dma_start(qt_sb[0:3, 1, :, :], qbias_d[1])
            with (
                tc.tile_pool(name="wbig", bufs=1) as wbigp,
            ):
                wo_sb = wbigp.tile([128, 8, M], BF16)
                for c in range(2):
                    nc.sync.dma_start(wo_sb[:, 4 * c:4 * c + 4, :],
                                      wo_d[:, 4 * c:4 * c + 4, :])

                with (
                    tc.tile_pool(name="psC2", bufs=2,
                                 space=bass.MemorySpace.PSUM) as psC2,
                    tc.tile_pool(name="psC1", bufs=1,
                                 space=bass.MemorySpace.PSUM) as psC1,
                    tc.tile_pool(name="psPV", bufs=1,
                                 space=bass.MemorySpace.PSUM) as psPV,
                    tc.tile_pool(name="psBC", bufs=1,
                                 space=bass.MemorySpace.PSUM) as psBC,
                ):
                    pv_pair = [None, None]
                    for s in range(16):
                        par, idx = s % 2, s // 2
                        h = PERM[s]
                        g = h // 4
                        assert g % 2 == par
                        gi = g // 2
                        pv = psPV.tile([128, SQ], F32, tag=f"pv{s % 2}",
                                       name=f"pv{s % 2}")
                        pv_pair[par] = pv

                        ps2a = psC2.tile([128, 1024], F32, tag="c2",
                                         name="ps2a")
                        ps2b = psC2.tile([128, 1024], F32, tag="c2",
                                         name="ps2b")
                        ps1 = psC1.tile([128, 512], F32, tag="c1",
                                        name="ps1")
                        chunks = ((CHUNK_C2A, ps2a), (CHUNK_C2B, ps2b),
                                  (CHUNK_C1, ps1))
                        for chunk, ps in chunks:
                            for kt, off in chunk:
                                span = QR[kt] - QL[kt]
                                nc.tensor.matmul(
                                    ps[:, off:off + span],
                                    kt_sb[:, par, gi,
                                          128 * kt:128 * kt + 128],
                                    qt_sb[:, par, idx, QL[kt]:QR[kt]],
                                    start=True, stop=True)
                        se2a = se2ap.tile([128, 1024], BF16, tag="se2a")
                        se2b = se2bp.tile([128, 1024], BF16, tag="se2b")
                        se1 = se1p.tile([128, 512], BF16, tag="se1")
                        semap = ((se2a, ps2a, MASKS_C2A),
                                 (se2b, ps2b, MASKS_C2B),
                                 (se1, ps1, MASKS_C1))
                        for se, ps, masks in semap:
                            nc.scalar.activation(se[:], ps[:], AF.Exp,
                                                 scale=0.125)
                            for off, mtype, eng in masks:
                                blk = se[:, off:off + 128]
                                if eng == 0:
                                    if mtype == 1:  # keep col' <= p
                                        nc.gpsimd.affine_select(
                                            blk, blk, pattern=[[-1, 128]],
                                            compare_op=mybir.AluOpType.is_ge,
                                            fill=0.0, base=0,
                                            channel_multiplier=1)
                                    else:  # keep col' >= p
                                        nc.gpsimd.affine_select(
                                            blk, blk, pattern=[[1, 128]],
                                            compare_op=mybir.AluOpType.is_ge,
                                            fill=0.0, base=0,
                                            channel_multiplier=-1)
                                else:
                                    nc.vector.tensor_mul(
                                        blk, blk,
                                        tri_sb[:, mtype - 1, :])
                        nmm = 0
                        sechunks = ((CHUNK_C2A, se2a), (CHUNK_C2B, se2b),
                                    (CHUNK_C1, se1))
                        for chunk, se in sechunks:
                            for kt, off in chunk:
                                span = QR[kt] - QL[kt]
                                nc.tensor.matmul(
                                    pv[0:65, QL[kt]:QR[kt]],
                                    v_sb[:, kt, g, 0:65],
                                    se[:, off:off + span],
                                    start=(nmm == 0), stop=(nmm == 7))
                                nmm += 1

                        if par == 1:
                            # finalize pair: attn = pv / denom per head
                            # (reciprocal runs after the broadcast on a
                            # [64, SQ] tile -- single-partition custom-DVE
                            # reciprocals misbehave on hardware)
                            p_i = idx
                            den = den0
                            nc.vector.tensor_copy(den[64:65, 0, :],
                                                  pv_pair[0][64:65, :])
                            nc.vector.tensor_copy(den[64:65, 1, :],
                                                  pv_pair[1][64:65, :])
                            bc0 = psBC.tile([64, SQ], F32, tag="bc",
                                            name="bc0")
                            nc.tensor.matmul(bc0[:], sel_sb[:, 1, 0:64],
                                             den[:, 0, :],
                                             start=True, stop=True)
                            bci0 = smallp.tile([64, SQ], F32, tag="bci0")
                            nc.vector.reciprocal_approx_fast(bci0[:],
                                                             bc0[:])
                            nc.vector.tensor_mul(attn_sb[0:64, p_i, :],
                                                 pv_pair[0][0:64, :],
                                                 bci0[:])
                            bc1 = psBC.tile([64, SQ], F32, tag="bc",
                                            name="bc1")
                            nc.tensor.matmul(bc1[:], sel_sb[:, 1, 0:64],
                                             den[:, 1, :],
                                             start=True, stop=True)
                            bci1 = smallp.tile([64, SQ], F32, tag="bci1")
                            nc.vector.reciprocal_approx_fast(bci1[:],
                                                             bc1[:])
                            atmp = smallp.tile([64, SQ], BF16, tag="atmp")
                            nc.vector.tensor_mul(atmp[:],
                                                 pv_pair[1][0:64, :],
                                                 bci1[:])
                            nc.sync.dma_start(attn_sb[64:128, p_i, :],
                                              atmp[:])

                # ---- output projection (wo still resident) ----
                outv = out_d.rearrange("(st p) m -> st p m", p=128)
                with tc.tile_pool(name="psO", bufs=4,
                                  space=bass.MemorySpace.PSUM) as psO:
                    for st in range(4):
                        osb = outp.tile([128, M], F32, tag="osb")
                        for mh in range(2):
                            op = psO.tile([128, 512], F32)
                            for ht in range(8):
                                nc.tensor.matmul(
                                    op[:],
                                    attn_sb[:, ht, 128 * st:128 * st + 128],
                                    wo_sb[:, ht, 512 * mh:512 * mh + 512],
                                    start=(ht == 0), stop=(ht == 7))
                            if mh == 0:
                                nc.scalar.copy(
                                    osb[:, 512 * mh:512 * mh + 512], op[:])
                            else:
                                nc.vector.tensor_copy(
                                    osb[:, 512 * mh:512 * mh + 512], op[:])
                        nc.sync.dma_start(outv[st], osb[:])

    if for_sim:
        nc.compile()
    else:
        nc.finalize()
    return nc


def make_in_maps(features, wq, wk, wv, wo, q_norm_w, k_norm_w):
    features = np.asarray(features, np.float32)
    wq = np.asarray(wq, np.float32)
    wk = np.asarray(wk, np.float32)
    wv = np.asarray(wv, np.float32)
    wo = np.asarray(wo, np.float32)
    q_norm_w = np.asarray(q_norm_w, np.float32)
    k_norm_w = np.asarray(k_norm_w, np.float32)

    # permute Q-head order (see PERM) in wq rows, q_norm_w, wo columns;
    # fold the rmsnorm weights into the projection rows (commutes with the
    # per-position rms scaling)
    wq_p = wq.reshape(H, D, M)[PERM].reshape(H * D, M)
    qnw_p = q_norm_w.reshape(H, D)[PERM].reshape(H * D)
    wq_p = wq_p * qnw_p[:, None]
    wk_f = wk * k_norm_w[:, None]
    wo_tp = wo.T.reshape(H, D, M)[PERM].reshape(H * D, M)  # wo.T rows = hd

    wq_pre = np.ascontiguousarray(
        wq_p.T.reshape(8, 128, 8, 128).transpose(2, 1, 0, 3)).astype(NPBF16)
    wk_pre = np.ascontiguousarray(
        wk_f.T.reshape(8, 128, 256).transpose(1, 0, 2)).astype(NPBF16)
    wv_pre = np.ascontiguousarray(
        wv.T.reshape(8, 128, 256).transpose(1, 0, 2)).astype(NPBF16)
    wo_pre = np.ascontiguousarray(
        wo_tp.reshape(8, 128, M).transpose(1, 0, 2)).astype(NPBF16)

    slopes = _alibi_slopes(H)

    vone = np.ones((128, 8, KV, 1), NPBF16)
    # row selectors: sel[:,0,:] picks partition 0, sel[:,1,:] picks 64
    sel = np.zeros((128, 2, 128), np.float32)
    sel[0, 0, :] = 1.0
    sel[64, 1, :] = 1.0
    # triangle mask tiles for the DVE mask-muls: tri[:,0]=keep col<=p,
    # tri[:,1]=keep col>=p
    p = np.arange(128)
    tri = np.zeros((128, 2, 128), NPBF16)
    tri[:, 0, :] = (p[None, :] <= p[:, None])
    tri[:, 1, :] = (p[None, :] >= p[:, None])

    # Q-side bias rows: rows 0/1 = 8*slope_h (pair with K-side pos_hi and
    # pos_lo; the position is split so both parts are bf16-exact),
    # row 2 = -8*slope_h*q - 320 (pairs with K-side ones; cancels in
    # softmax, keeps exp args <= ~-28 valid / < ~65 in masked triangles so
    # nothing overflows to inf before the mask-muls)
    qi = np.arange(SQ, dtype=np.float64)
    qbias = np.zeros((2, 3, 8, SQ), np.float32)
    for s in range(16):
        par, idx = s % 2, s // 2
        sl = slopes[PERM[s]]
        qbias[par, 0, idx, :] = 8.0 * sl
        qbias[par, 1, idx, :] = 8.0 * sl
        qbias[par, 2, idx, :] = -8.0 * sl * qi - 320.0
    qbias = qbias.astype(NPBF16)

    in_maps = []
    for b in range(B):
        for c in range(NCHUNK):
            q0 = c * SQ
            lo, hi = q0 - WIN, q0 + SQ
            fs = np.zeros((SK, M), np.float32)
            src_lo = max(lo, 0)
            fs[src_lo - lo:, :] = features[b, src_lo:hi, :]
            feat_pre = np.ascontiguousarray(
                fs.T.reshape(8, 128, SK).transpose(1, 0, 2)).astype(NPBF16)
            # K-side bias rows: rows 0/1 = pos_hi/pos_lo with
            # pos_hi + pos_lo = pos-512, both bf16-exact (halo positions
            # get a huge negative value so exp underflows to 0); row 2 = 1
            kbias = np.zeros((3, 2, SK), np.float32)
            pos = np.arange(SK, dtype=np.float64) - 512.0
            pos_hi = 4.0 * np.floor(pos / 4.0)
            pos_lo = pos - pos_hi
            if c == 0:
                pos_hi[:512] = -1e30
                pos_lo[:512] = 0.0
            kbias[0, :, :] = pos_hi[None, :]
            kbias[1, :, :] = pos_lo[None, :]
            kbias[2, :, :] = 1.0
            kbias = kbias.astype(NPBF16)
            in_maps.append({
                "feat": feat_pre, "wq": wq_pre, "wk": wk_pre, "wv": wv_pre,
                "wo": wo_pre,
                "onesin": np.ones((128, 128), np.float32),
                "sel": sel, "tri": tri, "vone": vone,
                "kbias": kbias, "qbias": qbias,
            })
    return in_maps


_NC_CACHE = {}


def kernel(features, wq, wk, wv, wo, q_norm_w, k_norm_w,
           num_heads=16, num_kv_heads=4, head_dim=64, sliding_window=512,
           **_unused):
    assert int(num_heads) == H and int(num_kv_heads) == KV
    assert int(head_dim) == D and int(sliding_window) == WIN
    from concourse.bass_utils import run_bass_kernel_spmd

    if "nc" not in _NC_CACHE:
        _NC_CACHE["nc"] = build_nc(for_sim=False)
    nc = _NC_CACHE["nc"]
    in_maps = make_in_maps(features, wq, wk, wv, wo, q_norm_w, k_norm_w)
    res = run_bass_kernel_spmd(nc, in_maps, core_ids=list(range(N_CORES)))
    outs = [r["out"] for r in res.results]
    full = np.stack(outs, axis=0).reshape(B, NCHUNK * SQ, M)
    return full.astype(np.float32)
